# revision 29
# baseline (speedup 1.0000x reference)
"""Multi-head causal attention on 8 TRN2 NeuronCores.

B=2, S=2048, D=1024, H=16 heads, head_dim=64. Tensor-parallel over heads:
core c owns heads {2c, 2c+1}. Each core:
  stage 1 (per 512-token piece): qT/kT/vT = W_c @ x.T (feature-major,
           bf16 matmuls, fp32 psum), then v -> token-major via PE
           transpose with a ones column per head appended (gives the
           softmax denominator for free).
  stage 2: attention in scoresT (k-major) layout, softmax without
           partition-axis reductions. Diagonal k-blocks use trimmed
           matmul streams (only unmasked q columns) plus one [128,128]
           triangular mask multiply per diagonal block on DVE.
           Tasks run as head-pairs (both heads of one (b, q-piece)) so
           the 64-contraction score matmuls of the two heads overlap in
           the PE array via row-group tiling. The ctx accumulation for
           group g runs one round behind its exp, decoupling ScalarE
           (exp) from TensorE.
  stage 3: partial output projection woven into the round stream,
           staged per piece and written with one DMA per piece.
DMA: x is shipped in a piece-contiguous host layout so each piece is
1-2 descriptors (2-8KB/partition lines) instead of 8 small ones, and
the output is staged to one DMA per piece; this keeps the Sync engine's
~0.6us/trigger cost off the critical path. The PE array is pre-warmed
with dense dummy matmuls during the initial DMA wait so the HAM clock
gate releases (1.2->2.4GHz) before real work, and the exp activation
table is preloaded concurrently. Host sums the 8 partial outputs and
adds the bias.
"""
import numpy as np
import ml_dtypes

B, S, D, H = 2, 2048, 1024, 16
HD = 64          # head dim
NT = B * S       # 4096 tokens
P = 128          # partitions
NCORES = 8
HPC = 2          # heads per core
NB = S // P      # 16 k-blocks per batch
NM = S // 512    # 4 q-pieces per batch
NP = NT // 512   # 8 token pieces overall
VCB = 2 * (HD + 1)   # 130: v block cols: h0 feats+1, h1 feats+1

_cache = {}


def _build():
    import concourse.bass as bass
    import concourse.mybir as mybir
    from concourse import bacc
    import concourse.tile as tile
    from concourse.masks import make_identity

    BF16 = mybir.dt.bfloat16
    F32 = mybir.dt.float32
    Exp = mybir.ActivationFunctionType.Exp

    nc = bacc.Bacc("TRN2", target_bir_lowering=False, debug=False,
                   num_devices=NCORES)

    # x: chunk-major [128, 16 chunks * 2048]; chunk (n, half) holds
    # cc-blocks 4*half..4*half+3 of piece n, 512 cols each
    xT_d = nc.dram_tensor("xT", [P, 16 * 2048], BF16, kind="ExternalInput")
    wq_d = nc.dram_tensor("wq", [P, D], BF16, kind="ExternalInput")
    wkvo_d = nc.dram_tensor("wkvo", [P, 3 * D], BF16, kind="ExternalInput")
    # out: piece-major [128, 8 pieces * 4096]; piece n holds f-blocks
    # 0..7 of cols [n*512, (n+1)*512)
    out_d = nc.dram_tensor("out", [P, NP * 4096], BF16,
                           kind="ExternalOutput")

    with tile.TileContext(nc) as tc:
        with tc.tile_pool(name="const", bufs=1) as const, \
             tc.tile_pool(name="xp", bufs=1) as xp, \
             tc.tile_pool(name="qk", bufs=1) as qk, \
             tc.tile_pool(name="misc", bufs=4) as misc, \
             tc.tile_pool(name="stg", bufs=2) as stg, \
             tc.tile_pool(name="pt", bufs=8) as ptp, \
             tc.tile_pool(name="pp", bufs=2, space="PSUM") as pp, \
             tc.tile_pool(name="sc", bufs=2, space="PSUM") as scp, \
             tc.tile_pool(name="cx", bufs=2, space="PSUM") as cxp:

            # ---- input DMAs, hoisted in dependency order ----
            wq = const.tile([P, D], BF16, tag="wq")
            wkvo = const.tile([P, 3 * D], BF16, tag="wkvo")
            x_all = xp.tile([P, 16 * 2048], BF16, tag="x")
            for ch in range(2):   # piece 0 first, before the weights
                nc.sync.dma_start(x_all[:, ch * 2048:(ch + 1) * 2048],
                                  xT_d.ap()[:, ch * 2048:(ch + 1) * 2048])
            nc.sync.dma_start(wq[:], wq_d.ap())
            nc.sync.dma_start(wkvo[:], wkvo_d.ap())
            for n in range(1, NP):   # whole pieces for the rest
                nc.sync.dma_start(x_all[:, n * 4096:(n + 1) * 4096],
                                  xT_d.ap()[:, n * 4096:(n + 1) * 4096])
            w_sb = {"wq": wq, "wk": wkvo[:, 0:D], "wv": wkvo[:, D:2 * D]}
            wo = wkvo[:, 2 * D:3 * D]

            # ---- constants built on-chip during the DMA wait ----
            # pre-warm the PE clock gate (HAM) with dense dummy matmuls so
            # real work runs at 2.4GHz; these execute while x is landing
            wsrc = misc.tile([P, 512], BF16, tag="wsrc", name="wsrc")
            nc.gpsimd.memset(wsrc[:], 0.5)
            for i in range(15):
                wd = pp.tile([P, 512], F32, tag="p1", name=f"wd{i}")
                nc.tensor.matmul(wd[:], wsrc[:, 0:P], wsrc[:],
                                 start=True, stop=True)

            ident = const.tile([P, P], BF16, tag="ident")
            make_identity(nc, ident[:])
            # [tri(128) | ones(512) | tri(128)]: one multiply masks both
            # diagonal windows of a (t, t+1) group, 640 cols apart
            trimask = const.tile([P, 768], BF16, tag="trimask")
            nc.gpsimd.memset(trimask[:], 1.0)
            for w0 in (0, 640):
                nc.gpsimd.affine_select(
                    out=trimask[:, w0:w0 + P],
                    in_=trimask[:, w0:w0 + P],
                    compare_op=mybir.AluOpType.is_ge,
                    fill=0.0, base=0,
                    # keep 1.0 where col >= partition (unmasked), else 0
                    pattern=[[1, P]], channel_multiplier=-1)

            # preload the exp table set while ScalarE is idle
            wtile = misc.tile([1, 8], F32, tag="wrm", name="wrm")
            nc.scalar.activation(wtile[:], wsrc[0:1, 0:8], Exp, scale=0.125)

            qT = qk.tile([P, NT], BF16, tag="qT")
            kT = qk.tile([P, NT], BF16, tag="kT")
            vT = qk.tile([P, NT], BF16, tag="vT")
            v_sb = qk.tile([P, (NT // P) * VCB], BF16, tag="v")
            nc.gpsimd.memset(v_sb[:], 1.0)
            ctxT = qk.tile([P, NT], BF16, tag="ctxT")
            # pre-zero pt ring and score psum so trimmed ops never read
            # uninitialized memory
            for i in range(8):
                pt0 = ptp.tile([P, 1024], BF16, tag="pt", name=f"ptz{i}")
                nc.gpsimd.memset(pt0[:], 0.0)
            for i in range(2):
                sc0 = scp.tile([P, 1024], F32, tag="sc", name=f"scz{i}")
                nc.vector.memset(sc0[:], 0.0)

            # ---- stage 1 sub-units ----
            s1_ps = {}

            def s1_proj_half(n, wname, dst, half):
                cols = slice(n * 512, (n + 1) * 512)
                w = w_sb[wname]
                if half == 0:
                    s1_ps[(wname, n)] = pp.tile([P, 512], F32, tag="p1",
                                                name=f"p1_{wname}_{n}")
                ps = s1_ps[(wname, n)]
                ch = 2 * n + half
                for cc4 in range(4):
                    cc = half * 4 + cc4
                    nc.tensor.matmul(
                        ps[:], w[:, cc * P:(cc + 1) * P],
                        x_all[:, ch * 2048 + cc4 * 512:
                              ch * 2048 + (cc4 + 1) * 512],
                        start=(cc == 0), stop=(cc == 7))
                if half == 1:
                    nc.vector.tensor_copy(dst[:, cols], ps[:])
                    del s1_ps[(wname, n)]

            def s1_vtrans(n, half):
                for t in range(4 * n + 2 * half, 4 * n + 2 * half + 2):
                    pst = pp.tile([P, P], BF16, tag="p1", name=f"ptr{t}")
                    nc.tensor.transpose(pst[:], vT[:, t * P:(t + 1) * P],
                                        ident[:])
                    dst3 = v_sb[:, t * VCB:(t + 1) * VCB].rearrange(
                        "p (h f) -> p h f", f=HD + 1)[:, :, 0:HD]
                    src3 = pst[:, :].rearrange("p (h f) -> p h f", f=HD)
                    nc.vector.tensor_copy(dst3, src3)

            q1 = []
            for n in range(NP):
                for wname, dst in (("wq", qT), ("wk", kT), ("wv", vT)):
                    for half in range(2):
                        q1.append((n, lambda n=n, w=wname, d=dst, h=half:
                                   s1_proj_half(n, w, d, h)))
                q1 += [(n, lambda n=n: s1_vtrans(n, 0)),
                       (n, lambda n=n: s1_vtrans(n, 1))]

            i1 = 0
            done1 = -1

            def pump_q1(need):
                nonlocal i1, done1
                while done1 < need and i1 < len(q1):
                    n, fn = q1[i1]
                    fn()
                    if i1 + 1 >= len(q1) or q1[i1 + 1][0] != n:
                        done1 = n
                    i1 += 1

            def drip_q1(k):
                nonlocal i1, done1
                for _ in range(k):
                    if i1 >= len(q1):
                        return
                    n, fn = q1[i1]
                    fn()
                    if i1 + 1 >= len(q1) or q1[i1 + 1][0] != n:
                        done1 = n
                    i1 += 1

            # ---- stage 3: per (piece, f-block) units, staged output ----
            st_tiles = {}
            st_count = {}
            s3q = []

            def s3_unit(n, f, flush=False):
                if n not in st_tiles:
                    st_tiles[n] = stg.tile([P, 4096], BF16, tag="st",
                                           name=f"st{n}")
                    st_count[n] = 0
                st = st_tiles[n]
                # during the drain the scores pool is dead: rotate flush
                # psum through it too, doubling MM/copy overlap depth
                pool = scp if (flush and f % 2 == 0) else pp
                pso = pool.tile([P, 512], F32, tag="sc" if pool is scp
                                else "p1", name=f"p3_{f}_{n}")
                nc.tensor.matmul(pso[:], wo[:, f * P:(f + 1) * P],
                                 ctxT[:, n * 512:(n + 1) * 512],
                                 start=True, stop=True)
                if flush and f % 2 == 0:
                    # ScalarE is idle during the drain; split the copies
                    # across both engines to double flush throughput
                    nc.scalar.copy(st[:, f * 512:(f + 1) * 512], pso[:])
                else:
                    nc.vector.tensor_copy(st[:, f * 512:(f + 1) * 512],
                                          pso[:])
                st_count[n] += 1
                if flush:
                    # overlap the final transfers with the remaining copies
                    nc.sync.dma_start(
                        out_d.ap()[:, n * 4096 + f * 512:
                                   n * 4096 + (f + 1) * 512],
                        st[:, f * 512:(f + 1) * 512])
                    if st_count[n] == 8:
                        del st_tiles[n]
                elif st_count[n] == 8:
                    nc.sync.dma_start(
                        out_d.ap()[:, n * 4096:(n + 1) * 4096], st[:])
                    del st_tiles[n]

            def drip_s3(k):
                for _ in range(k):
                    if not s3q:
                        return
                    s3_unit(*s3q.pop(0))

            # ---- stage 2: head-pair tasks, ctx one round behind ----
            class Pair:
                def __init__(self, b, m):
                    self.b, self.m = b, m
                    self.njs = 4 * m + 4
                    self.G = self.njs // 2
                    self.scs = {}
                    self.pts = {}
                    self.cx = [cxp.tile([HD + 1, 512], F32, tag="cx",
                                        name=f"cx_{b}_{m}_{hl}")
                               for hl in range(HPC)]

                def req(self, g):
                    return self.b * NM + max(self.m, (2 * g + 1) // 4)

                def scores(self, g):
                    b, m = self.b, self.m
                    for hl in range(HPC):
                        self.scs[hl] = scp.tile([P, 1024], F32, tag="sc",
                                                name=f"sc_{b}_{m}_{g}_{hl}")
                    qc0 = b * S + m * 512
                    for t2 in range(2):
                        j = 2 * g + t2
                        trim = max(0, 128 * (j - 4 * m))
                        kc0 = b * S + j * P
                        for hl in range(HPC):
                            hb = hl * HD
                            nc.tensor.matmul(
                                self.scs[hl][:, t2 * 512 + trim:
                                             (t2 + 1) * 512],
                                kT[hb:hb + HD, kc0:kc0 + P],
                                qT[hb:hb + HD, qc0 + trim:qc0 + 512],
                                start=True, stop=True,
                                tile_position=(hb, 0))

                def exp(self, g):
                    b, m = self.b, self.m
                    t = 2 * g - 4 * m
                    for hl in range(HPC):
                        pt = ptp.tile([P, 1024], BF16, tag="pt",
                                      name=f"pt_{b}_{m}_{g}_{hl}")
                        self.pts[(g, hl)] = pt
                        if t >= 2:
                            off = 128 * t
                            nc.scalar.activation(
                                pt[:].rearrange("p (a c) -> p a c",
                                                a=2)[:, :, off:512],
                                self.scs[hl][:].rearrange(
                                    "p (a c) -> p a c", a=2)[:, :, off:512],
                                Exp, scale=0.125)
                        else:
                            nc.scalar.activation(pt[:], self.scs[hl][:],
                                                 Exp, scale=0.125)
                        if t >= 0:
                            # both diagonal windows sit 640 cols apart at
                            # [128t, 128t+128) and [640+128t, 768+128t)
                            w0 = 128 * t
                            nc.vector.tensor_mul(
                                pt[:, w0:w0 + 768],
                                pt[:, w0:w0 + 768], trimask[:])

                def ctx(self, g):
                    b, m = self.b, self.m
                    for t2 in range(2):
                        j = 2 * g + t2
                        trim = max(0, 128 * (j - 4 * m))
                        for hl in range(HPC):
                            pt = self.pts[(g, hl)]
                            vb = (b * NB + j) * VCB + hl * 65
                            nc.tensor.matmul(
                                self.cx[hl][:, trim:512],
                                v_sb[:, vb:vb + HD + 1],
                                pt[:, t2 * 512 + trim:(t2 + 1) * 512],
                                start=(j == 0), stop=(j == self.njs - 1))
                    for hl in range(HPC):
                        del self.pts[(g, hl)]

                def normalize(self):
                    b, m = self.b, self.m
                    qc0 = b * S + m * 512
                    for hl in range(HPC):
                        hb = hl * HD
                        sm = misc.tile([1, 512], F32, tag="sm",
                                       name=f"sm_{b}_{m}_{hl}")
                        nc.vector.tensor_copy(sm[:],
                                              self.cx[hl][HD:HD + 1, :])
                        rc = misc.tile([1, 512], F32, tag="rc",
                                       name=f"rc_{b}_{m}_{hl}")
                        nc.vector.reciprocal_approx_fast(rc[:], sm[:])
                        bc = misc.tile([HD, 512], F32, tag="bc",
                                       name=f"bc_{b}_{m}_{hl}")
                        nc.gpsimd.partition_broadcast(bc[:], rc[:])
                        nc.vector.tensor_mul(
                            ctxT[hb:hb + HD, qc0:qc0 + 512],
                            self.cx[hl][0:HD, :], bc[:])

            order = [(0, 0), (0, 1), (0, 2), (0, 3),
                     (1, 1), (1, 0), (1, 2), (1, 3)]
            pending = []   # (pair, group) ctx units, run 2 rounds behind

            def pop_ctx():
                Tp, gp = pending.pop(0)
                Tp.ctx(gp)
                if gp == Tp.G - 1:
                    Tp.normalize()
                    n = Tp.b * NM + Tp.m
                    s3q.extend((n, f) for f in range(8))

            rnd = 0
            for (b, m) in order:
                T = Pair(b, m)
                pump_q1(b * NM + m)
                for g in range(T.G):
                    pump_q1(T.req(g))
                    T.scores(g)
                    if len(pending) >= 2:
                        pop_ctx()
                    drip_q1(1 + rnd % 2)
                    drip_s3(2)
                    T.exp(g)
                    pending.append((T, g))
                    rnd += 1
            while pending:
                pop_ctx()
                drip_s3(2)
            pump_q1(NP)
            # catch-up DMAs for pieces partially staged before the flush
            # (flush units DMA per f-block; earlier copies never were)
            for n, st in st_tiles.items():
                done = st_count[n]
                if done:
                    nc.sync.dma_start(
                        out_d.ap()[:, n * 4096:n * 4096 + done * 512],
                        st[:, 0:done * 512])
            while s3q:
                s3_unit(*s3q.pop(0), flush=True)
    nc.compile()
    return nc


def _get_nc():
    if "nc" not in _cache:
        _cache["nc"] = _build()
    return _cache["nc"]


def _bf16(a):
    return np.ascontiguousarray(a).astype(ml_dtypes.bfloat16)


def _prepare_in_maps(x, Wq, Wk, Wv, Wo):
    # x -> chunk-major layout: [p, (n, half, cc4, col)]
    xr = np.asarray(x, np.float32).reshape(NP, 512, 8, P)  # n, col, cc, p
    xT = _bf16(xr.transpose(3, 0, 2, 1).reshape(P, 16 * 2048))

    def wlayout(Wslice):  # [128 feats, 1024 d] -> [p, cc*128+f]
        return _bf16(Wslice.reshape(P, 8, P).transpose(2, 1, 0)
                     .reshape(P, D))

    in_maps = []
    for c in range(NCORES):
        rows = slice(c * P, (c + 1) * P)
        wk = wlayout(np.asarray(Wk, np.float32)[rows, :])
        wv = wlayout(np.asarray(Wv, np.float32)[rows, :])
        wo = _bf16(np.asarray(Wo, np.float32)[:, rows].T)
        in_maps.append({
            "xT": xT,
            "wq": wlayout(np.asarray(Wq, np.float32)[rows, :]),
            "wkvo": np.concatenate([wk, wv, wo], axis=1),
        })
    return in_maps


def _run(inputs, trace=False, tmpdir=None):
    from concourse.bass_utils import run_bass_kernel_spmd
    nc = _get_nc()
    in_maps = _prepare_in_maps(inputs["x"], inputs["Wq"], inputs["Wk"],
                               inputs["Wv"], inputs["Wo"])
    res = run_bass_kernel_spmd(nc, in_maps, core_ids=list(range(NCORES)),
                               trace=trace, tmpdir=tmpdir)
    acc = np.zeros((D, NT), np.float32)
    for r in res.results:
        o = r["out"].astype(np.float32).reshape(P, NP, 8, 512)
        acc += o.transpose(2, 0, 1, 3).reshape(D, NT)
    out = acc.T.reshape(B, S, D) + np.asarray(inputs["bo"], np.float32)
    return out.astype(np.float32), res


def kernel(**inputs):
    out, _ = _run(inputs)
    return out


def kernel_traced(tmpdir=None, **inputs):
    out, res = _run(inputs, trace=True, tmpdir=tmpdir)
    return out, res
